# revision 11
# baseline (speedup 1.0000x reference)
"""Cuboid (windowed) self-attention Trainium2 kernel.

x (2, 8, 64, 64, 256) -> 128 windows of (512 tokens, 256 ch); per window:
qkv = xw @ qkv_w, 8-head softmax attention (dh=32), proj + bias; reverse.

Sharding: data-parallel over the 128 windows across 8 NeuronCores
(16 windows/core), weights replicated.

Per-core dataflow (layouts chosen so no on-device transposes are needed;
softmax is max-free since logits are ~N(0,1) -> exp overflow impossible):

  host pre-transposes each window to xT (256 ch, 512 tok)
  QT/KT = Wq/Wk.T @ xT         (head-dim on partitions, tokens free)
  V     = xT.T @ Wv            (tokens on partitions, head-dim free)
  S^T   = K @ Q^T              per head (keys on partitions), PE row-packed
                               2 heads at a time (contraction is only 32)
  P^T   = exp(scale * S^T)     one ScalarE activation per (head-pair,
                               key-chunk), PSUM->SBUF, 1024 wide
                               == THE throughput bottleneck (~14us/window)
  outT/denom = [V | 1].T @ P^T col-packed (2 PV + 2 denom matmuls share
               the PE array), accumulated over the 4 key chunks
  normalize: reciprocal of denom rows + broadcast-matmul (constant E
             replicates each head's 1/denom row across its 32 partitions)
  y = outT.T @ proj_w + b      (queries on partitions) -> DMA out

Emission is software-pipelined across windows: PE/ACT/DVE are in-order
engines and PSUM is only 8 banks, so window w+1's QT/KT matmuls are
emitted mid-window-w (right after head-group 0's denominators retire
their PSUM bank) to keep ScalarE's exp stream gapless at the window
boundary.
"""

import numpy as np

import concourse.bass as bass
import concourse.bacc as bacc
import concourse.tile as tile
from concourse import mybir
from concourse.bass_utils import run_bass_kernel_spmd

B, T, H, W, C = 2, 8, 64, 64, 256
HEADS = 8
WS = 8
DH = C // HEADS            # 32
N = T * WS * WS            # 512 tokens per window
NWIN = B * (H // WS) * (W // WS)   # 128
NCORES = 8
WPC = NWIN // NCORES       # 16 windows per core
SCALE = DH ** -0.5
P = 128

F32 = mybir.dt.float32
F32R = mybir.dt.float32r
BF16 = mybir.dt.bfloat16
EXP = mybir.ActivationFunctionType.Exp


def _emit(ctx, tc, nwin, adt, qdt, x_d, wqkv_d, wproj_d, bias_d, out_d):
    nc = tc.nc

    singles = ctx.enter_context(tc.tile_pool(name="singles", bufs=1))
    xpool = ctx.enter_context(tc.tile_pool(name="xpool", bufs=2))
    qkpool = ctx.enter_context(tc.tile_pool(name="qkpool", bufs=2))
    ppool = ctx.enter_context(tc.tile_pool(name="ppool", bufs=3))
    opool = ctx.enter_context(tc.tile_pool(name="opool", bufs=2))
    ypool = ctx.enter_context(tc.tile_pool(name="ypool", bufs=2))
    mpool = ctx.enter_context(tc.tile_pool(name="mpool", bufs=2))
    ps2 = ctx.enter_context(tc.tile_pool(name="ps2", bufs=2, space="PSUM"))
    ps1 = ctx.enter_context(tc.tile_pool(name="ps1", bufs=4, space="PSUM"))

    # ---- constants ----
    wqkv = singles.tile([P, 2, 3 * C], qdt)   # [:, cc, j]: channel chunk cc
    nc.sync.dma_start(wqkv, wqkv_d.rearrange("(g p) j -> p g j", p=P))
    wproj = singles.tile([P, 2, C], adt)
    if adt == qdt:
        nc.sync.dma_start(wproj, wproj_d.rearrange("(g p) j -> p g j", p=P))
    else:
        wproj_f = singles.tile([P, 2, C], qdt)
        nc.sync.dma_start(wproj_f, wproj_d.rearrange("(g p) j -> p g j", p=P))
        nc.vector.tensor_copy(wproj, wproj_f)
    bias = singles.tile([P, C], F32)
    nc.sync.dma_start(
        bias,
        bass.AP(tensor=bias_d.tensor, offset=bias_d.offset, ap=[[0, P], [1, C]]),
    )
    ones = singles.tile([P, 32], adt)
    nc.vector.memset(ones, 1.0)

    st = [None] * nwin   # per-window live tiles

    def start_window(w):
        s = {"s": {}, "p": {}, "pv": {}, "d": {}}
        s["xt"] = xpool.tile([P, 2, N], qdt, tag="xt", name="xt")
        nc.sync.dma_start(s["xt"], x_d[w].rearrange("(g p) n -> p g n", p=P))
        s["qt"] = qkpool.tile([P, 2, N], adt, tag="qt", name="qt")
        s["kt"] = qkpool.tile([P, 2, N], adt, tag="kt", name="kt")
        s["vv"] = qkpool.tile([P, 2, N], adt, tag="vv", name="vv")
        s["outT"] = opool.tile([P, 2, N], adt, tag="outT", name="outT")
        st[w] = s

    def qkv_qk(w, g):
        # Q^T,K^T head-group g: out = W[:, cols].T @ xT
        s = st[w]
        for name, base in (("qt", 0), ("kt", 256)):
            acc = ps1.tile([P, N], F32, tag="b1", name="acc")
            for cc in range(2):
                nc.tensor.matmul(
                    acc,
                    lhsT=wqkv[:, cc, base + 128 * g:base + 128 * (g + 1)],
                    rhs=s["xt"][:, cc, :],
                    start=(cc == 0), stop=(cc == 1))
            nc.vector.tensor_copy(s[name][:, g, :], acc)

    def qkv_v(w):
        # V: out = xT.T @ Wv   (tokens on partitions)
        s = st[w]
        for half in range(2):
            acc = ps1.tile([P, N], F32, tag="b1", name="acc")
            for sub in range(2):     # token chunk m = 2*half + sub
                m = 2 * half + sub
                for cc in range(2):
                    nc.tensor.matmul(
                        acc[:, sub * C:(sub + 1) * C],
                        lhsT=s["xt"][:, cc, 128 * m:128 * (m + 1)],
                        rhs=wqkv[:, cc, 512:768],
                        start=(cc == 0), stop=(cc == 1))
            nc.vector.tensor_copy(s["vv"][:, half, :], acc)

    def s_step(w, step):
        # S^T for (head pair, key chunk): keys on partitions, queries free
        s = st[w]
        pair, j = step // 4, step % 4
        g = pair // 2
        sp = ps2.tile([P, 2, N], F32, tag="s", name="sp")
        for hl in range(2):
            row = 64 * (pair % 2) + 32 * hl
            nc.tensor.matmul(
                sp[:, hl, :],
                lhsT=s["kt"][row:row + 32, g, 128 * j:128 * (j + 1)],
                rhs=s["qt"][row:row + 32, g, :],
                start=True, stop=True,
                tile_position=(row, 0))
        s["s"][step] = sp

    def exp_step(w, step):
        s = st[w]
        pT = ppool.tile([P, 2, N], adt, tag="pT", name="pT")
        nc.scalar.activation(pT, s["s"].pop(step), EXP, scale=SCALE)
        s["p"][step] = pT

    def pvd_step(w, step):
        # col-packed: 2 PV matmuls + 2 denominator matmuls, accum over j
        s = st[w]
        pair, j = step // 4, step % 4
        g = pair // 2
        if step % 8 == 0:
            s["pv"][g] = ps1.tile([P, N], F32, tag="b1", name="pv")
            s["d"][g] = ps1.tile([P, N], F32, tag="b1", name="d")
        pT = s["p"].pop(step)
        for hl in range(2):
            h = 2 * pair + hl
            hh = h % 4
            nc.tensor.matmul(
                s["pv"][g][32 * hh:32 * hh + 32, :],
                lhsT=s["vv"][:, j // 2, (j % 2) * C + 32 * h:(j % 2) * C + 32 * h + 32],
                rhs=pT[:, hl, :],
                start=(j == 0), stop=(j == 3),
                tile_position=(0, 32 * hh), skip_group_check=True)
            cd = (32 * hh + 64) % 128
            nc.tensor.matmul(
                s["d"][g][cd:cd + 32, :],
                lhsT=ones,
                rhs=pT[:, hl, :],
                start=(j == 0), stop=(j == 3),
                tile_position=(0, cd), skip_group_check=True)

    def finish_g(w, g):
        # 1/denom, un-rotating the +64-partition shift the D col-groups used
        # (D lands at (32*hh+64)%128 so it can pack with PV in the PE array)
        s = st[w]
        d = s["d"].pop(g)
        rc = mpool.tile([P, N], F32, tag="rc", name="rc")
        nc.vector.reciprocal(rc[0:64], d[64:128])
        nc.vector.reciprocal(rc[64:128], d[0:64])
        nc.vector.tensor_mul(s["outT"][:, g, :], s["pv"].pop(g), rc)

    def tail(w):
        s = st[w]
        y = ypool.tile([P, 4, C], F32, tag="y")
        for half in range(2):
            acc = ps1.tile([P, N], F32, tag="b1", name="acc")
            for sub in range(2):
                m = 2 * half + sub
                for g in range(2):
                    nc.tensor.matmul(
                        acc[:, sub * C:(sub + 1) * C],
                        lhsT=s["outT"][:, g, 128 * m:128 * (m + 1)],
                        rhs=wproj[:, g, :],
                        start=(g == 0), stop=(g == 1))
            for sub in range(2):
                nc.vector.tensor_add(y[:, 2 * half + sub, :],
                                     acc[:, sub * C:(sub + 1) * C], bias)
        nc.sync.dma_start(out_d[w].rearrange("(m p) c -> p m c", p=P), y)
        st[w] = None

    # ---- pipelined emission ----
    start_window(0)
    qkv_qk(0, 0)
    qkv_qk(0, 1)
    qkv_v(0)
    s_step(0, 0)
    s_step(0, 1)
    for w in range(nwin):
        for step in range(16):
            exp_step(w, step)
            t = step + 2
            if t < 16:
                s_step(w, t)
            elif w + 1 < nwin:
                s_step(w + 1, t - 16)
            pvd_step(w, step)
            if step == 7:
                finish_g(w, 0)
                if w + 1 < nwin:
                    start_window(w + 1)
                    qkv_qk(w + 1, 0)
        finish_g(w, 1)
        if w + 1 < nwin:
            qkv_v(w + 1)
            qkv_qk(w + 1, 1)
        tail(w)


def _build_bass(nwin: int, adt, qdt) -> bass.Bass:
    nc = bacc.Bacc("TRN2", target_bir_lowering=False)
    x_d = nc.declare_dram_parameter("xt", [nwin, C, N], F32R, isOutput=False)
    wqkv_d = nc.declare_dram_parameter("qkv_w", [C, 3 * C], F32R, isOutput=False)
    wproj_d = nc.declare_dram_parameter("proj_w", [C, C], F32R, isOutput=False)
    bias_d = nc.declare_dram_parameter("proj_b", [C], F32, isOutput=False)
    out_d = nc.declare_dram_parameter("out", [nwin, N, C], F32, isOutput=True)
    from contextlib import ExitStack
    with tile.TileContext(nc) as tc, ExitStack() as ctx:
        _emit(ctx, tc, nwin, adt, qdt, x_d.ap(), wqkv_d.ap(), wproj_d.ap(),
              bias_d.ap(), out_d.ap())
    nc.compile()
    return nc


_CACHE: dict = {}
DTS = {"bf16": BF16, "f32r": F32R, "f32": F32}


def get_nc(nwin=WPC, attn="bf16", qkv="f32r"):
    key = (nwin, attn, qkv)
    if key not in _CACHE:
        _CACHE[key] = _build_bass(nwin, DTS[attn], DTS[qkv])
    return _CACHE[key]


def shard_inputs(x, qkv_w, proj_w, proj_b, wpc=WPC):
    hn, wn = H // WS, W // WS
    xw = np.asarray(x, dtype=np.float32).reshape(B, T, hn, WS, wn, WS, C)
    xw = xw.transpose(0, 2, 4, 1, 3, 5, 6).reshape(NWIN, N, C)
    xT = np.ascontiguousarray(xw.transpose(0, 2, 1))
    return [
        {
            "xt": xT[i * wpc:(i + 1) * wpc],
            "qkv_w": np.asarray(qkv_w, dtype=np.float32),
            "proj_w": np.asarray(proj_w, dtype=np.float32),
            "proj_b": np.asarray(proj_b, dtype=np.float32),
        }
        for i in range(NCORES)
    ]


def unshard(results):
    y = np.concatenate([np.asarray(results[i]["out"]) for i in range(NCORES)],
                       axis=0)
    hn, wn = H // WS, W // WS
    y = y.reshape(B, hn, wn, T, WS, WS, C)
    y = y.transpose(0, 3, 1, 4, 2, 5, 6).reshape(B, T, H, W, C)
    return np.ascontiguousarray(y.astype(np.float32))


def kernel(x, qkv_w, proj_w, proj_b):
    nc = get_nc()
    in_maps = shard_inputs(x, qkv_w, proj_w, proj_b)
    res = run_bass_kernel_spmd(nc, in_maps, list(range(NCORES))).results
    return unshard(res)


if __name__ == "__main__":
    rng = np.random.default_rng(0)
    x = rng.standard_normal((B, T, H, W, C), dtype=np.float32)
    qkv_w = (rng.standard_normal((C, 3 * C), dtype=np.float32) * C ** -0.5)
    proj_w = (rng.standard_normal((C, C), dtype=np.float32) * C ** -0.5)
    proj_b = np.zeros((C,), dtype=np.float32)
    y = kernel(x, qkv_w, proj_w, proj_b)
    print(y.shape, y.dtype)


# revision 17
# speedup vs baseline: 1.1927x; 1.1927x over previous
"""Cuboid (windowed) self-attention Trainium2 kernel.

x (2, 8, 64, 64, 256) -> 128 windows of (512 tokens, 256 ch); per window:
qkv = xw @ qkv_w, 8-head softmax attention (dh=32), proj + bias; reverse.

Sharding: data-parallel over the 128 windows across 8 NeuronCores
(16 windows/core), weights replicated.

Per-core dataflow (layouts chosen so no on-device transposes are needed;
softmax is max-free since logits are ~N(0,1) -> exp overflow impossible):

  host pre-transposes each window to xT (256 ch, 512 tok)
  QT/KT = Wq/Wk.T @ xT         (head-dim on partitions, tokens free)
  V     = xT.T @ Wv            (tokens on partitions, head-dim free)
  S^T   = K @ Q^T              per head (keys on partitions), PE row-packed
                               2 heads at a time (contraction is only 32)
  P^T   = exp(scale * S^T)     one ScalarE activation per (head-pair,
                               key-chunk), PSUM->SBUF, 1024 wide
                               == THE throughput bottleneck (~14us/window)
  outT/denom = [V | 1].T @ P^T col-packed (2 PV + 2 denom matmuls share
               the PE array), accumulated over the 4 key chunks
  normalize: reciprocal of denom rows + broadcast-matmul (constant E
             replicates each head's 1/denom row across its 32 partitions)
  y = outT.T @ proj_w + b      (queries on partitions) -> DMA out

Emission is software-pipelined across windows: PE/ACT/DVE are in-order
engines and PSUM is only 8 banks, so window w+1's QT/KT matmuls are
emitted mid-window-w (right after head-group 0's denominators retire
their PSUM bank) to keep ScalarE's exp stream gapless at the window
boundary.
"""

import numpy as np

import concourse.bass as bass
import concourse.bacc as bacc
import concourse.tile as tile
from concourse import mybir
from concourse.bass_utils import run_bass_kernel_spmd

B, T, H, W, C = 2, 8, 64, 64, 256
HEADS = 8
WS = 8
DH = C // HEADS            # 32
N = T * WS * WS            # 512 tokens per window
NWIN = B * (H // WS) * (W // WS)   # 128
NCORES = 8
WPC = NWIN // NCORES       # 16 windows per core
SCALE = DH ** -0.5
P = 128

F32 = mybir.dt.float32
F32R = mybir.dt.float32r
BF16 = mybir.dt.bfloat16
EXP = mybir.ActivationFunctionType.Exp


def _build_emat() -> np.ndarray:
    """E[p, m] = 1 iff p == 32*(m//32): bc = E.T @ rc replicates head hh's
    1/denom row (rc partition 32*hh) over its 32 outT partitions."""
    e = np.zeros((128, 128), dtype=np.float32)
    for hh in range(4):
        e[32 * hh, 32 * hh:32 * hh + 32] = 1.0
    return e


def _emit(ctx, tc, nwin, adt, qdt, x_d, wqkv_d, wproj_d, bias_d, emat_d, zr_d,
          out_d,
          reps=1,
          variant=""):
    nc = tc.nc

    singles = ctx.enter_context(tc.tile_pool(name="singles", bufs=1))
    xpool = ctx.enter_context(tc.tile_pool(name="xpool", bufs=2))
    qkpool = ctx.enter_context(tc.tile_pool(name="qkpool", bufs=2))
    ppool = ctx.enter_context(tc.tile_pool(name="ppool", bufs=3))
    opool = ctx.enter_context(tc.tile_pool(name="opool", bufs=2))
    ypool = ctx.enter_context(tc.tile_pool(name="ypool", bufs=2))
    mpool = ctx.enter_context(tc.tile_pool(name="mpool", bufs=2))
    ps2 = ctx.enter_context(tc.tile_pool(name="ps2", bufs=2, space="PSUM"))
    ps1 = ctx.enter_context(tc.tile_pool(name="ps1", bufs=4, space="PSUM"))

    # ---- constants ----
    wqkv = singles.tile([P, 2, 3 * C], qdt)   # [:, cc, j]: channel chunk cc
    nc.sync.dma_start(wqkv, wqkv_d.rearrange("(g p) j -> p g j", p=P))
    wproj = singles.tile([P, 2, C], adt)
    if adt == qdt:
        nc.sync.dma_start(wproj, wproj_d.rearrange("(g p) j -> p g j", p=P))
    else:
        wproj_f = singles.tile([P, 2, C], qdt)
        nc.sync.dma_start(wproj_f, wproj_d.rearrange("(g p) j -> p g j", p=P))
        nc.vector.tensor_copy(wproj, wproj_f)
    bias = singles.tile([P, C], F32)
    nc.sync.dma_start(
        bias,
        bass.AP(tensor=bias_d.tensor, offset=bias_d.offset, ap=[[0, P], [1, C]]),
    )
    emat = singles.tile([P, P], qdt)
    nc.sync.dma_start(emat, emat_d)
    # rc rows {0,32,64,96} are rewritten per window with 1/denom; all other
    # rows are zeroed once here (E's zero rows meet finite zeros, not junk).
    # f32r memset is invalid ISA, so zero-fill via a broadcast DMA instead.
    rc = singles.tile([P, 2, N], qdt)
    nc.sync.dma_start(
        rc,
        bass.AP(tensor=zr_d.tensor, offset=zr_d.offset,
                ap=[[0, P], [N, 2], [1, N]]),
    )

    st = [None] * nwin   # per-window live tiles

    def start_window(w):
        s = {"s": {}, "p": {}, "pv": {}, "d": {}}
        s["xt"] = xpool.tile([P, 2, N], qdt, tag="xt", name="xt")
        nc.sync.dma_start(s["xt"], x_d[w].rearrange("(g p) n -> p g n", p=P))
        s["qt"] = qkpool.tile([P, 2, N], adt, tag="qt", name="qt")
        s["kt"] = qkpool.tile([P, 2, N], adt, tag="kt", name="kt")
        s["vv"] = qkpool.tile([P, 4, 264], adt, tag="vv", name="vv")
        nc.vector.memset(s["vv"][:, :, 32:264:33], 1.0)
        s["outT"] = opool.tile([P, 2, N], adt, tag="outT", name="outT")
        st[w] = s

    def qkv_qk(w, g):
        # Q^T,K^T head-group g: out = W[:, cols].T @ xT
        s = st[w]
        for name, base in (("qt", 0), ("kt", 256)):
            acc = ps1.tile([P, N], F32, tag="b1", name="acc")
            for cc in range(2):
                nc.tensor.matmul(
                    acc,
                    lhsT=wqkv[:, cc, base + 128 * g:base + 128 * (g + 1)],
                    rhs=s["xt"][:, cc, :],
                    start=(cc == 0), stop=(cc == 1))
            nc.vector.tensor_copy(s[name][:, g, :], acc)

    def qkv_v(w):
        # V: out = xT.T @ Wv   (tokens on partitions)
        s = st[w]
        for half in range(2):
            acc = ps1.tile([P, N], F32, tag="b1", name="acc")
            for sub in range(2):     # token chunk m = 2*half + sub
                m = 2 * half + sub
                for cc in range(2):
                    nc.tensor.matmul(
                        acc[:, sub * C:(sub + 1) * C],
                        lhsT=s["xt"][:, cc, 128 * m:128 * (m + 1)],
                        rhs=wqkv[:, cc, 512:768],
                        start=(cc == 0), stop=(cc == 1))
            nc.vector.tensor_copy(
                s["vv"][:, 2 * half:2 * half + 2, :]
                    .rearrange("p a (h c) -> p a h c", h=8)[:, :, :, 0:32],
                acc.rearrange("p (a h c) -> p a h c", a=2, h=8))

    def s_step(w, step):
        # S^T for (head pair, key chunk): keys on partitions, queries free
        s = st[w]
        pair, j = step // 4, step % 4
        g = pair // 2
        sp = ps2.tile([P, 2, N], F32, tag="s", name="sp")
        for hl in range(2):
            row = 64 * (pair % 2) + 32 * hl
            nc.tensor.matmul(
                sp[:, hl, :],
                lhsT=s["kt"][row:row + 32, g, 128 * j:128 * (j + 1)],
                rhs=s["qt"][row:row + 32, g, :],
                start=True, stop=True,
                tile_position=(row, 0))
        s["s"][step] = sp

    def exp_step(w, step):
        s = st[w]
        pT = ppool.tile([P, 2, N], adt, tag="pT", name="pT")
        nc.scalar.activation(pT, s["s"].pop(step), EXP, scale=SCALE)
        s["p"][step] = pT

    def pvd_step(w, step):
        # augmented PV: lhsT = [V_h | ones] (M=33) -> rows 64*hl..+32 are the
        # head's outT^T rows, row 64*hl+32 is its softmax denominator
        s = st[w]
        pair, j = step // 4, step % 4
        if j == 0:
            s["pv"][pair] = ps1.tile([P, N], F32, tag="b1", name="pv")
        pT = s["p"].pop(step)
        for hl in range(2):
            h = 2 * pair + hl
            nc.tensor.matmul(
                s["pv"][pair][64 * hl:64 * hl + 33, :],
                lhsT=s["vv"][:, j, 33 * h:33 * h + 33],
                rhs=pT[:, hl, :],
                start=(j == 0), stop=(j == 3),
                tile_position=(0, 64 * hl), skip_group_check=True)

    def finish_g(w, g):
        # per head: 1/denom row -> rc; one E-matmul broadcasts all four
        # heads' recips over their 32-row blocks; multiply PV rows into outT
        s = st[w]
        pairs = (2 * g, 2 * g + 1)
        with nc.allow_low_precision(reason="f32r is a full 32-bit container"):
            for pi, pair in enumerate(pairs):
                for hl in range(2):
                    hh = 2 * pi + hl
                    nc.vector.reciprocal(
                        rc[32 * hh:32 * hh + 1, g, :],
                        s["pv"][pair][64 * hl + 32:64 * hl + 33, :])
        bc_ps = ps1.tile([P, N], F32, tag="b1", name="bc_ps")
        nc.tensor.matmul(bc_ps, lhsT=emat, rhs=rc[:, g, :],
                         start=True, stop=True)
        bc = mpool.tile([P, N], F32, tag="bc", name="bc")
        nc.vector.tensor_copy(bc, bc_ps)
        for pi, pair in enumerate(pairs):
            pv = s["pv"][pair] if pair in s["pv"] else None
            for hl in range(2):
                hh = 2 * pi + hl
                nc.vector.tensor_mul(
                    s["outT"][32 * hh:32 * hh + 32, g, :],
                    pv[64 * hl:64 * hl + 32, :],
                    bc[32 * hh:32 * hh + 32, :])
            del s["pv"][pair]

    def tail(w):
        s = st[w]
        y = ypool.tile([P, 4, C], F32, tag="y")
        for half in range(2):
            acc = ps1.tile([P, N], F32, tag="b1", name="acc")
            for sub in range(2):
                m = 2 * half + sub
                for g in range(2):
                    nc.tensor.matmul(
                        acc[:, sub * C:(sub + 1) * C],
                        lhsT=s["outT"][:, g, 128 * m:128 * (m + 1)],
                        rhs=wproj[:, g, :],
                        start=(g == 0), stop=(g == 1))
            for sub in range(2):
                nc.vector.tensor_add(y[:, 2 * half + sub, :],
                                     acc[:, sub * C:(sub + 1) * C], bias)
        nc.sync.dma_start(out_d[w].rearrange("(m p) c -> p m c", p=P), y)
        st[w] = None

    # ---- pipelined emission ----
    def one_pass():
        start_window(0)
        qkv_qk(0, 0)
        qkv_qk(0, 1)
        qkv_v(0)
        s_step(0, 0)
        s_step(0, 1)
        for w in range(nwin):
            for step in range(16):
                exp_step(w, step)
                t = step + 2
                if t < 16:
                    s_step(w, t)
                elif w + 1 < nwin:
                    s_step(w + 1, t - 16)
                pvd_step(w, step)
                if step == 7:
                    finish_g(w, 0)
                    if w + 1 < nwin:
                        start_window(w + 1)
                        qkv_qk(w + 1, 0)
            finish_g(w, 1)
            if w + 1 < nwin:
                qkv_v(w + 1)
                qkv_qk(w + 1, 1)
            tail(w)

    if reps == 1:
        one_pass()
    else:
        # device-side repeat loop, for timing: isolates kernel time from
        # the ~90ms axon dispatch overhead
        with tc.For_i(0, reps, 1):
            one_pass()


def _build_bass(nwin: int, adt, qdt, reps: int = 1, variant: str = "") -> bass.Bass:
    nc = bacc.Bacc("TRN2", target_bir_lowering=False)
    x_d = nc.declare_dram_parameter("xt", [nwin, C, N], F32R, isOutput=False)
    wqkv_d = nc.declare_dram_parameter("qkv_w", [C, 3 * C], F32R, isOutput=False)
    wproj_d = nc.declare_dram_parameter("proj_w", [C, C], F32R, isOutput=False)
    bias_d = nc.declare_dram_parameter("proj_b", [C], F32, isOutput=False)
    emat_d = nc.declare_dram_parameter("emat", [128, 128], F32R, isOutput=False)
    zr_d = nc.declare_dram_parameter("zr", [2, 512], F32R, isOutput=False)
    out_d = nc.declare_dram_parameter("out", [nwin, N, C], F32, isOutput=True)
    from contextlib import ExitStack
    with tile.TileContext(nc) as tc, ExitStack() as ctx:
        _emit(ctx, tc, nwin, adt, qdt, x_d.ap(), wqkv_d.ap(), wproj_d.ap(),
              bias_d.ap(), emat_d.ap(), zr_d.ap(), out_d.ap(), reps=reps,
              variant=variant)
    nc.compile()
    return nc


_CACHE: dict = {}
DTS = {"bf16": BF16, "f32r": F32R, "f32": F32}


def get_nc(nwin=WPC, attn="bf16", qkv="f32r", reps=1, variant=""):
    key = (nwin, attn, qkv, reps, variant)
    if key not in _CACHE:
        _CACHE[key] = _build_bass(nwin, DTS[attn], DTS[qkv], reps=reps,
                                  variant=variant)
    return _CACHE[key]


def shard_inputs(x, qkv_w, proj_w, proj_b, wpc=WPC):
    hn, wn = H // WS, W // WS
    xw = np.asarray(x, dtype=np.float32).reshape(B, T, hn, WS, wn, WS, C)
    xw = xw.transpose(0, 2, 4, 1, 3, 5, 6).reshape(NWIN, N, C)
    xT = np.ascontiguousarray(xw.transpose(0, 2, 1))
    return [
        {
            "xt": xT[i * wpc:(i + 1) * wpc],
            "qkv_w": np.asarray(qkv_w, dtype=np.float32),
            "proj_w": np.asarray(proj_w, dtype=np.float32),
            "proj_b": np.asarray(proj_b, dtype=np.float32),
            "emat": _build_emat(),
            "zr": np.zeros((2, 512), dtype=np.float32),
        }
        for i in range(NCORES)
    ]


def unshard(results):
    y = np.concatenate([np.asarray(results[i]["out"]) for i in range(NCORES)],
                       axis=0)
    hn, wn = H // WS, W // WS
    y = y.reshape(B, hn, wn, T, WS, WS, C)
    y = y.transpose(0, 3, 1, 4, 2, 5, 6).reshape(B, T, H, W, C)
    return np.ascontiguousarray(y.astype(np.float32))


def kernel(x, qkv_w, proj_w, proj_b):
    nc = get_nc()
    in_maps = shard_inputs(x, qkv_w, proj_w, proj_b)
    res = run_bass_kernel_spmd(nc, in_maps, list(range(NCORES))).results
    return unshard(res)


if __name__ == "__main__":
    rng = np.random.default_rng(0)
    x = rng.standard_normal((B, T, H, W, C), dtype=np.float32)
    qkv_w = (rng.standard_normal((C, 3 * C), dtype=np.float32) * C ** -0.5)
    proj_w = (rng.standard_normal((C, C), dtype=np.float32) * C ** -0.5)
    proj_b = np.zeros((C,), dtype=np.float32)
    y = kernel(x, qkv_w, proj_w, proj_b)
    print(y.shape, y.dtype)


# revision 21
# speedup vs baseline: 171.2747x; 143.6011x over previous
"""Cuboid (windowed) self-attention Trainium2 kernel.

x (2, 8, 64, 64, 256) -> 128 windows of (512 tokens, 256 ch); per window:
qkv = xw @ qkv_w, 8-head softmax attention (dh=32), proj + bias; reverse.

Sharding: data-parallel over the 128 windows across 8 NeuronCores
(16 windows/core), weights replicated.

Per-core dataflow (layouts chosen so no on-device transposes are needed;
softmax is max-free since logits are ~N(0,1) -> exp overflow impossible):

  host pre-transposes each window to xT (256 ch, 512 tok)
  QT/KT = Wq/Wk.T @ xT         (head-dim on partitions, tokens free)
  V     = xT.T @ Wv            (tokens on partitions, head-dim free)
  S^T   = K @ Q^T              per head (keys on partitions), PE row-packed
                               2 heads at a time (contraction is only 32)
  P^T   = exp(scale * S^T)     one ScalarE activation per (head-pair,
                               key-chunk), PSUM->SBUF, 1024 wide
                               == THE throughput bottleneck (~14us/window)
  outT/denom = [V | 1].T @ P^T col-packed (2 PV + 2 denom matmuls share
               the PE array), accumulated over the 4 key chunks
  normalize: reciprocal of denom rows + broadcast-matmul (constant E
             replicates each head's 1/denom row across its 32 partitions)
  y = outT.T @ proj_w + b      (queries on partitions) -> DMA out

Emission is software-pipelined across windows: PE/ACT/DVE are in-order
engines and PSUM is only 8 banks, so window w+1's QT/KT matmuls are
emitted mid-window-w (right after head-group 0's denominators retire
their PSUM bank) to keep ScalarE's exp stream gapless at the window
boundary.
"""

import numpy as np

import concourse.bass as bass
import concourse.bacc as bacc
import concourse.tile as tile
from concourse import mybir
from concourse.bass_utils import run_bass_kernel_spmd

B, T, H, W, C = 2, 8, 64, 64, 256
HEADS = 8
WS = 8
DH = C // HEADS            # 32
N = T * WS * WS            # 512 tokens per window
NWIN = B * (H // WS) * (W // WS)   # 128
NCORES = 8
WPC = NWIN // NCORES       # 16 windows per core
SCALE = DH ** -0.5
P = 128

F32 = mybir.dt.float32
F32R = mybir.dt.float32r
BF16 = mybir.dt.bfloat16
EXP = mybir.ActivationFunctionType.Exp


def _emit(ctx, tc, nwin, adt, qdt, x_d, wqkv_d, wproj_d, bias_d, out_d,
          reps=1,
          variant=""):
    nc = tc.nc

    singles = ctx.enter_context(tc.tile_pool(name="singles", bufs=1))
    xpool = ctx.enter_context(tc.tile_pool(name="xpool", bufs=2))
    qkpool = ctx.enter_context(tc.tile_pool(name="qkpool", bufs=2))
    ppool = ctx.enter_context(tc.tile_pool(name="ppool", bufs=3))
    opool = ctx.enter_context(tc.tile_pool(name="opool", bufs=2))
    ypool = ctx.enter_context(tc.tile_pool(name="ypool", bufs=2))
    mpool = ctx.enter_context(tc.tile_pool(name="mpool", bufs=2))
    ps2 = ctx.enter_context(tc.tile_pool(name="ps2", bufs=2, space="PSUM"))
    ps1 = ctx.enter_context(tc.tile_pool(name="ps1", bufs=4, space="PSUM"))

    # ---- constants ----
    wqkv = singles.tile([P, 2, 3 * C], qdt)   # [:, cc, j]: channel chunk cc
    nc.sync.dma_start(wqkv, wqkv_d.rearrange("(g p) j -> p g j", p=P))
    wproj = singles.tile([P, 2, C], adt)
    if adt == qdt:
        nc.sync.dma_start(wproj, wproj_d.rearrange("(g p) j -> p g j", p=P))
    else:
        wproj_f = singles.tile([P, 2, C], qdt)
        nc.sync.dma_start(wproj_f, wproj_d.rearrange("(g p) j -> p g j", p=P))
        nc.vector.tensor_copy(wproj, wproj_f)
    bias = singles.tile([P, C], F32)
    nc.sync.dma_start(
        bias,
        bass.AP(tensor=bias_d.tensor, offset=bias_d.offset, ap=[[0, P], [1, C]]),
    )
    ones = singles.tile([P, 32], adt)
    nc.vector.memset(ones, 1.0)

    def cd_of(hh):
        return (32 * hh + 64) % 128

    st = [None] * nwin   # per-window live tiles

    def start_window(w):
        s = {"s": {}, "p": {}, "pv": {}, "d": {}}
        s["xt"] = xpool.tile([P, 2, N], qdt, tag="xt", name="xt")
        nc.sync.dma_start(s["xt"], x_d[w].rearrange("(g p) n -> p g n", p=P))
        s["qt"] = qkpool.tile([P, 2, N], adt, tag="qt", name="qt")
        s["kt"] = qkpool.tile([P, 2, N], adt, tag="kt", name="kt")
        s["vv"] = qkpool.tile([P, 2, N], adt, tag="vv", name="vv")
        s["outT"] = opool.tile([P, 2, N], adt, tag="outT", name="outT")
        st[w] = s

    def qkv_qk(w, g):
        # Q^T,K^T head-group g: out = W[:, cols].T @ xT
        s = st[w]
        for name, base in (("qt", 0), ("kt", 256)):
            acc = ps1.tile([P, N], F32, tag="b1", name="acc")
            for cc in range(2):
                nc.tensor.matmul(
                    acc,
                    lhsT=wqkv[:, cc, base + 128 * g:base + 128 * (g + 1)],
                    rhs=s["xt"][:, cc, :],
                    start=(cc == 0), stop=(cc == 1))
            nc.vector.tensor_copy(s[name][:, g, :], acc)

    def qkv_v(w):
        # V: out = xT.T @ Wv   (tokens on partitions)
        s = st[w]
        for half in range(2):
            acc = ps1.tile([P, N], F32, tag="b1", name="acc")
            for sub in range(2):     # token chunk m = 2*half + sub
                m = 2 * half + sub
                for cc in range(2):
                    nc.tensor.matmul(
                        acc[:, sub * C:(sub + 1) * C],
                        lhsT=s["xt"][:, cc, 128 * m:128 * (m + 1)],
                        rhs=wqkv[:, cc, 512:768],
                        start=(cc == 0), stop=(cc == 1))
            nc.vector.tensor_copy(s["vv"][:, half, :], acc)

    def s_step(w, step):
        # S^T for (head pair, key chunk): keys on partitions, queries free
        s = st[w]
        pair, j = step // 4, step % 4
        g = pair // 2
        sp = ps2.tile([P, 2, N], F32, tag="s", name="sp")
        for hl in range(2):
            row = 64 * (pair % 2) + 32 * hl
            nc.tensor.matmul(
                sp[:, hl, :],
                lhsT=s["kt"][row:row + 32, g, 128 * j:128 * (j + 1)],
                rhs=s["qt"][row:row + 32, g, :],
                start=True, stop=True,
                tile_position=(row, 0))
        s["s"][step] = sp

    def exp_step(w, step):
        s = st[w]
        pT = ppool.tile([P, 2, N], adt, tag="pT", name="pT")
        nc.scalar.activation(pT, s["s"].pop(step), EXP, scale=SCALE)
        s["p"][step] = pT

    def pvd_step(w, step):
        # col-packed: 2 PV matmuls + 2 denominator matmuls, accum over j
        s = st[w]
        pair, j = step // 4, step % 4
        g = pair // 2
        if step % 8 == 0:
            s["pv"][g] = ps1.tile([P, N], F32, tag="b1", name="pv")
            s["d"][g] = ps1.tile([P, N], F32, tag="b1", name="d")
        pT = s["p"].pop(step)
        for hl in range(2):
            h = 2 * pair + hl
            hh = h % 4
            nc.tensor.matmul(
                s["pv"][g][32 * hh:32 * hh + 32, :],
                lhsT=s["vv"][:, j // 2, (j % 2) * C + 32 * h:(j % 2) * C + 32 * h + 32],
                rhs=pT[:, hl, :],
                start=(j == 0), stop=(j == 3),
                tile_position=(0, 32 * hh), skip_group_check=True)
            nc.tensor.matmul(
                s["d"][g][cd_of(hh):cd_of(hh) + 32, :],
                lhsT=ones,
                rhs=pT[:, hl, :],
                start=(j == 0), stop=(j == 3),
                tile_position=(0, cd_of(hh)), skip_group_check=True)

    def finish_g(w, g):
        # 1/denom, un-rotating the +64-partition shift the D col-groups use
        # (D lands at (32*hh+64)%128 so it can pack with PV in the PE array)
        s = st[w]
        d = s["d"].pop(g)
        rc = mpool.tile([P, N], F32, tag="rc", name="rc")
        nc.vector.reciprocal(rc[0:64], d[64:128])
        nc.vector.reciprocal(rc[64:128], d[0:64])
        nc.vector.tensor_mul(s["outT"][:, g, :], s["pv"].pop(g), rc)

    def tail(w):
        s = st[w]
        y = ypool.tile([P, 4, C], F32, tag="y")
        for half in range(2):
            acc = ps1.tile([P, N], F32, tag="b1", name="acc")
            for sub in range(2):
                m = 2 * half + sub
                for g in range(2):
                    nc.tensor.matmul(
                        acc[:, sub * C:(sub + 1) * C],
                        lhsT=s["outT"][:, g, 128 * m:128 * (m + 1)],
                        rhs=wproj[:, g, :],
                        start=(g == 0), stop=(g == 1))
            for sub in range(2):
                nc.vector.tensor_add(y[:, 2 * half + sub, :],
                                     acc[:, sub * C:(sub + 1) * C], bias)
        nc.sync.dma_start(out_d[w].rearrange("(m p) c -> p m c", p=P), y)
        st[w] = None

    # ---- pipelined emission ----
    def one_pass():
        start_window(0)
        qkv_qk(0, 0)
        qkv_qk(0, 1)
        qkv_v(0)
        s_step(0, 0)
        s_step(0, 1)
        for w in range(nwin):
            for step in range(16):
                exp_step(w, step)
                t = step + 2
                if t < 16:
                    s_step(w, t)
                elif w + 1 < nwin:
                    s_step(w + 1, t - 16)
                pvd_step(w, step)
                if step == 7:
                    finish_g(w, 0)
                    if w + 1 < nwin:
                        start_window(w + 1)
                        qkv_qk(w + 1, 0)
            finish_g(w, 1)
            if w + 1 < nwin:
                qkv_v(w + 1)
                qkv_qk(w + 1, 1)
            tail(w)

    if reps == 1:
        one_pass()
    else:
        # device-side repeat loop, for timing: isolates kernel time from
        # the ~90ms axon dispatch overhead
        with tc.For_i(0, reps, 1):
            one_pass()


def _build_bass(nwin: int, adt, qdt, reps: int = 1, variant: str = "") -> bass.Bass:
    nc = bacc.Bacc("TRN2", target_bir_lowering=False)
    x_d = nc.declare_dram_parameter("xt", [nwin, C, N], F32R, isOutput=False)
    wqkv_d = nc.declare_dram_parameter("qkv_w", [C, 3 * C], F32R, isOutput=False)
    wproj_d = nc.declare_dram_parameter("proj_w", [C, C], F32R, isOutput=False)
    bias_d = nc.declare_dram_parameter("proj_b", [C], F32, isOutput=False)
    out_d = nc.declare_dram_parameter("out", [nwin, N, C], F32, isOutput=True)
    from contextlib import ExitStack
    with tile.TileContext(nc) as tc, ExitStack() as ctx:
        _emit(ctx, tc, nwin, adt, qdt, x_d.ap(), wqkv_d.ap(), wproj_d.ap(),
              bias_d.ap(), out_d.ap(), reps=reps, variant=variant)
    nc.compile()
    return nc


_CACHE: dict = {}
DTS = {"bf16": BF16, "f32r": F32R, "f32": F32}


def get_nc(nwin=WPC, attn="bf16", qkv="f32r", reps=1, variant=""):
    key = (nwin, attn, qkv, reps, variant)
    if key not in _CACHE:
        _CACHE[key] = _build_bass(nwin, DTS[attn], DTS[qkv], reps=reps,
                                  variant=variant)
    return _CACHE[key]


def shard_inputs(x, qkv_w, proj_w, proj_b, wpc=WPC):
    hn, wn = H // WS, W // WS
    xw = np.asarray(x, dtype=np.float32).reshape(B, T, hn, WS, wn, WS, C)
    xw = xw.transpose(0, 2, 4, 1, 3, 5, 6).reshape(NWIN, N, C)
    xT = np.ascontiguousarray(xw.transpose(0, 2, 1))
    return [
        {
            "xt": xT[i * wpc:(i + 1) * wpc],
            "qkv_w": np.asarray(qkv_w, dtype=np.float32),
            "proj_w": np.asarray(proj_w, dtype=np.float32),
            "proj_b": np.asarray(proj_b, dtype=np.float32),
        }
        for i in range(NCORES)
    ]


def unshard(results):
    y = np.concatenate([np.asarray(results[i]["out"]) for i in range(NCORES)],
                       axis=0)
    hn, wn = H // WS, W // WS
    y = y.reshape(B, hn, wn, T, WS, WS, C)
    y = y.transpose(0, 3, 1, 4, 2, 5, 6).reshape(B, T, H, W, C)
    return np.ascontiguousarray(y.astype(np.float32))


def kernel(x, qkv_w, proj_w, proj_b):
    nc = get_nc()
    in_maps = shard_inputs(x, qkv_w, proj_w, proj_b)
    res = run_bass_kernel_spmd(nc, in_maps, list(range(NCORES))).results
    return unshard(res)


if __name__ == "__main__":
    rng = np.random.default_rng(0)
    x = rng.standard_normal((B, T, H, W, C), dtype=np.float32)
    qkv_w = (rng.standard_normal((C, 3 * C), dtype=np.float32) * C ** -0.5)
    proj_w = (rng.standard_normal((C, C), dtype=np.float32) * C ** -0.5)
    proj_b = np.zeros((C,), dtype=np.float32)
    y = kernel(x, qkv_w, proj_w, proj_b)
    print(y.shape, y.dtype)
